# revision 18
# baseline (speedup 1.0000x reference)
"""Trainium2 Bass kernel for nn_PredictionNetwork (LTC network).

Network: x[256,2048,5] -> flatten [256,10240] -> LTC cell A (n_in=10240, n_u=32,
6 ODE unfolds) -> LTC cell B (n_in=32, n_u=1, 6 unfolds) -> sigmoid -> [256].

Strategy (8 NeuronCores, single NEFF, SPMD with per-core input values):
  - Sensory stage of cell A dominates (84M sigmoid evals + weighted reductions
    over n_in). Shard the 32 units across cores (4 units/core); every core sees
    all 256 examples so the ACT engine gets 256-wide free dims.
  - Layout: partitions = n_in (80 tiles x 128), free = batch. Host pre-transposes
    x to [10240, 256] bf16. Per (i-tile, unit): DVE tensor_scalar computes
    z = x*A - C (input affine + synapse affine folded on host, per-partition
    scalars), ACT runs one big sigmoid per 16 units [128, 4096], PE reduces over
    n_in with stationary weights [sW*serev | sW] accumulating into one PSUM tile.
  - AllGather the [8,256] per-core partial sums; each core then extracts its
    32-example slice via a selection matmul (per-core 0/1 matrix input keeps the
    NEFF identical across cores) and runs the 6-step recurrence + cell B for its
    slice. Final [32] per core is concatenated on the host.
"""

import numpy as np
import ml_dtypes

import concourse.bacc as bacc
import concourse.bass as bass
import concourse.mybir as mybir
import concourse.tile as tile
from concourse.bass_utils import run_bass_kernel_spmd

BF16 = ml_dtypes.bfloat16
dt = mybir.dt
AF = mybir.ActivationFunctionType
ALU = mybir.AluOpType

N_CORES = 8
B = 256                  # batch
NIN = 10240              # seq*feat = cell A n_in
NU = 32                  # cell A units
UPC = NU // N_CORES      # units per core = 4
BPC = B // N_CORES       # batch slice per core = 32
NIT = NIN // 128         # 80 i-tiles
ICG = 4                  # i-tiles per chunk
NCHUNK = NIT // ICG      # 20 chunks
UNFOLDS = 6
ELAPSED = 1.0


def build_program(debug=()):
    """Build the Bass program. debug: iterable of stage names to emit as extra
    outputs ("red", "wsel", "h")."""
    nc = bacc.Bacc("TRN2", target_bir_lowering=False, debug=False,
                   num_devices=N_CORES)

    d_xq = nc.dram_tensor("xq", [NCHUNK, 128, ICG, B], dt.bfloat16,
                          kind="ExternalInput")
    d_asc = nc.dram_tensor("asc", [128, NIT, UPC], dt.float32, kind="ExternalInput")
    d_csc = nc.dram_tensor("csc", [128, NIT, UPC], dt.float32, kind="ExternalInput")
    d_w12 = nc.dram_tensor("w12", [128, NIT, UPC, 2 * UPC], dt.bfloat16,
                           kind="ExternalInput")
    d_rep4 = nc.dram_tensor("rep4", [NU, 128], dt.float32, kind="ExternalInput")
    d_wseln = nc.dram_tensor("wseln", [128, 8, NU], dt.bfloat16, kind="ExternalInput")
    d_wseld = nc.dram_tensor("wseld", [128, 8, NU], dt.bfloat16, kind="ExternalInput")
    d_sigv = nc.dram_tensor("sigv", [128, 8], dt.float32, kind="ExternalInput")
    d_msigv = nc.dram_tensor("msigv", [128, 8], dt.float32, kind="ExternalInput")
    d_cmt = nc.dram_tensor("cmt", [NU, 1], dt.float32, kind="ExternalInput")
    d_glvl = nc.dram_tensor("glvl", [NU, 1], dt.float32, kind="ExternalInput")
    d_dc = nc.dram_tensor("dc", [NU, 1], dt.float32, kind="ExternalInput")
    d_selh = nc.dram_tensor("selh", [2, 128, BPC], dt.float32, kind="ExternalInput")
    d_eye = nc.dram_tensor("eye32", [NU, NU], dt.float32, kind="ExternalInput")
    d_ab = nc.dram_tensor("ab", [NU, 1], dt.float32, kind="ExternalInput")
    d_cb = nc.dram_tensor("cb", [NU, 1], dt.float32, kind="ExternalInput")
    d_w12b = nc.dram_tensor("w12b", [NU, 2], dt.bfloat16, kind="ExternalInput")
    d_bsc = nc.dram_tensor("bsc", [1, 8], dt.float32, kind="ExternalInput")
    d_out = nc.dram_tensor("out", [1, BPC], dt.float32, kind="ExternalOutput")

    dbg = {}
    if "red" in debug:
        dbg["red"] = nc.dram_tensor("dbg_red", [2 * UPC, B], dt.float32,
                                    kind="ExternalOutput")
    if "wsel" in debug:
        dbg["wsel"] = nc.dram_tensor("dbg_wsel", [2, NU, BPC], dt.float32,
                                     kind="ExternalOutput")
    if "h" in debug:
        dbg["h"] = nc.dram_tensor("dbg_h", [NU, BPC], dt.float32,
                                  kind="ExternalOutput")

    with tile.TileContext(nc) as tc:
        with (
            tc.tile_pool(name="par", bufs=1) as par,
            tc.tile_pool(name="xp", bufs=4) as xp,
            tc.tile_pool(name="zp", bufs=3) as zp,
            tc.tile_pool(name="sp", bufs=3) as sp,
            tc.tile_pool(name="wk", bufs=1) as wk,
            tc.tile_pool(name="dram", bufs=1, space="DRAM") as dram,
        ):
            # ---- parameter loads ----
            asc = par.tile([128, NIT, UPC], dt.float32)
            csc = par.tile([128, NIT, UPC], dt.float32)
            w12 = par.tile([128, NIT, UPC, 2 * UPC], dt.bfloat16)
            nc.gpsimd.dma_start(asc[:], d_asc[:])
            nc.gpsimd.dma_start(csc[:], d_csc[:])
            nc.gpsimd.dma_start(w12[:], d_w12[:])
            rep4 = par.tile([NU, 128], dt.float32)
            wseln = par.tile([128, 8, NU], dt.bfloat16)
            wseld = par.tile([128, 8, NU], dt.bfloat16)
            sigv = par.tile([128, 8], dt.float32)
            msigv = par.tile([128, 8], dt.float32)
            cmt = par.tile([NU, 1], dt.float32)
            glvl = par.tile([NU, 1], dt.float32)
            dc = par.tile([NU, 1], dt.float32)
            selh0 = par.tile([128, BPC], dt.float32)
            selh1 = par.tile([128, BPC], dt.float32)
            eye32 = par.tile([NU, NU], dt.float32)
            ab = par.tile([NU, 1], dt.float32)
            cb = par.tile([NU, 1], dt.float32)
            w12b = par.tile([NU, 2], dt.bfloat16)
            bsc = par.tile([1, 8], dt.float32)
            for t, dr in ((rep4, d_rep4), (wseln, d_wseln), (wseld, d_wseld),
                          (sigv, d_sigv), (msigv, d_msigv), (cmt, d_cmt),
                          (glvl, d_glvl), (dc, d_dc),
                          (ab, d_ab), (cb, d_cb), (w12b, d_w12b), (bsc, d_bsc)):
                nc.gpsimd.dma_start(t[:], dr[:])
            nc.gpsimd.dma_start(selh0[:], d_selh[0])
            nc.gpsimd.dma_start(selh1[:], d_selh[1])
            nc.gpsimd.dma_start(eye32[:], d_eye[:])

            # warm the sigmoid table set while the first x chunk is in flight
            warm = wk.tile([1, 8], dt.float32)
            nc.scalar.activation(warm[:], bsc[:], AF.Sigmoid)

            # ---- sensory stage of cell A ----
            # two half-range accumulators so the first AllGather overlaps the
            # second half of the compute
            agos = []
            with tc.tile_pool(name="psA", bufs=1, space="PSUM") as psA:
                ps8h = [psA.tile([2 * UPC, B], dt.float32, tag=f"ps8_{h}",
                                 name=f"ps8_{h}")
                        for h in range(2)]
                for ic in range(NCHUNK):
                    half = ic // (NCHUNK // 2)
                    ps8 = ps8h[half]
                    icl = ic % (NCHUNK // 2)
                    x4 = xp.tile([128, ICG, B], dt.bfloat16)
                    nc.sync.dma_start(x4[:], d_xq[ic][:])
                    # last unit of each chunk goes through the fused-ACT path
                    # (sigmoid with per-partition scale/bias) to balance the
                    # DVE and ACT engines; the rest through DVE tensor_scalar
                    # + one big-tile sigmoid. csc holds -C so both paths add.
                    z = zp.tile([128, ICG * UPC, B], dt.bfloat16)
                    for t in range(ICG):
                        it = ICG * ic + t
                        for u in range(UPC):
                            if t * UPC + u == ICG * UPC - 1:
                                continue
                            nc.vector.tensor_scalar(
                                z[:, t * UPC + u, :], x4[:, t, :],
                                asc[:, it, u:u + 1], csc[:, it, u:u + 1],
                                ALU.mult, ALU.add)
                    s = sp.tile([128, ICG * UPC, B], dt.bfloat16)
                    nc.scalar.activation(s[:, 0:ICG * UPC - 1, :],
                                         z[:, 0:ICG * UPC - 1, :], AF.Sigmoid)
                    itl, ul = ICG * ic + ICG - 1, UPC - 1
                    nc.scalar.activation(s[:, ICG * UPC - 1, :],
                                         x4[:, ICG - 1, :], AF.Sigmoid,
                                         bias=csc[:, itl, ul:ul + 1],
                                         scale=asc[:, itl, ul:ul + 1])
                    for t in range(ICG):
                        it = ICG * ic + t
                        for u in range(UPC):
                            nc.tensor.matmul(
                                ps8[:], w12[:, it, u, :], s[:, t * UPC + u, :],
                                start=(icl == 0 and t == 0 and u == 0),
                                stop=(icl == NCHUNK // 2 - 1 and t == ICG - 1
                                      and u == UPC - 1))
                    if ic % (NCHUNK // 2) == NCHUNK // 2 - 1:
                        red = wk.tile([2 * UPC, B], dt.float32, tag=f"red_{half}")
                        nc.vector.tensor_copy(red[:], ps8[:])
                        agin = dram.tile([2 * UPC, B], dt.float32,
                                         tag=f"agin_{half}")
                        ago = dram.tile([N_CORES, 2 * UPC, B], dt.float32,
                                        addr_space="Shared", tag=f"ago_{half}")
                        nc.sync.dma_start(agin[:], red[:])
                        nc.gpsimd.collective_compute(
                            "AllGather", ALU.bypass,
                            replica_groups=[list(range(N_CORES))],
                            ins=[agin[:].opt()], outs=[ago[:].opt()])
                        agos.append(ago)

            with tc.tile_pool(name="psR", bufs=1, space="PSUM") as psR:
                # ---- iteration 0 of the cell A recurrence, AG-independent
                # part: v0 = 0, so the sigmoid + reduction matmuls can run
                # while the second AllGather is still in flight. The psum
                # groups stay open (stop=False) until the nm_pre/dcw inject
                # matmuls close them below.
                v = wk.tile([NU, BPC], dt.float32)
                nc.vector.memset(v[:], 0.0)
                pV = psR.tile([128, BPC], dt.float32, tag="pV", name="pV0")
                nc.tensor.matmul(pV[:], rep4[:], v[:], start=True, stop=True)
                sA = wk.tile([128, 8, BPC], dt.bfloat16, tag="sA", name="sA0")
                for jt in range(8):
                    nc.scalar.activation(sA[:, jt, :], pV[:], AF.Sigmoid,
                                         bias=msigv[:, jt:jt + 1],
                                         scale=sigv[:, jt:jt + 1])
                pn = psR.tile([NU, BPC], dt.float32, tag="pn", name="pn0")
                pd = psR.tile([NU, BPC], dt.float32, tag="pd", name="pd0")
                for jt in range(8):
                    nc.tensor.matmul(pn[:], wseln[:, jt, :], sA[:, jt, :],
                                     start=(jt == 0), stop=False)
                for jt in range(8):
                    nc.tensor.matmul(pd[:], wseld[:, jt, :], sA[:, jt, :],
                                     start=(jt == 0), stop=False)

                # ---- sum the two half-range AllGathers: [j, b] layout
                # (ago flat addr = j*512 + kind*256 + b with j = src*4+u)
                wsum = []
                for kk in range(2):
                    wa = wk.tile([NU, B], dt.float32, tag=f"wa_{kk}",
                                 name=f"wa_{kk}")
                    wb = wk.tile([NU, B], dt.float32, tag=f"wb_{kk}",
                                 name=f"wb_{kk}")
                    va = agos[0][:].rearrange("s (u k) b -> k (s u) b", u=UPC, k=2)
                    vb = agos[1][:].rearrange("s (u k) b -> k (s u) b", u=UPC, k=2)
                    nc.sync.dma_start(wa[:], va[kk])
                    nc.sync.dma_start(wb[:], vb[kk])
                    ws = wk.tile([NU, B], dt.float32, tag=f"ws_{kk}",
                                 name=f"ws_{kk}")
                    nc.vector.tensor_tensor(ws[:], wa[:], wb[:], ALU.add)
                    wsum.append(ws)
                if "red" in dbg:
                    nc.sync.dma_start(dbg["red"][:], wsum[0][:])

                # transpose [j, B] -> two [128, j] halves (PE), then per-core
                # slice: wns[j, b_loc] = sum_b wT[b, j] * sel[b, b_loc]
                with (
                    tc.tile_pool(name="psT", bufs=1, space="PSUM") as psT,
                    tc.tile_pool(name="psSel", bufs=1, space="PSUM") as psSel,
                ):
                    pwn = psSel.tile([NU, BPC], dt.float32, tag="pwn")
                    pwd = psSel.tile([NU, BPC], dt.float32, tag="pwd")
                    for kk, pw in ((0, pwn), (1, pwd)):
                        for h, sel in ((0, selh0), (1, selh1)):
                            pT = psT.tile([128, NU], dt.float32, tag="pT",
                                          name=f"pT_{kk}_{h}")
                            nc.tensor.transpose(
                                pT[:], wsum[kk][:, 128 * h:128 * (h + 1)],
                                eye32[:])
                            wT = wk.tile([128, NU], dt.float32, tag="wT",
                                         name=f"wT_{kk}_{h}")
                            nc.vector.tensor_copy(wT[:], pT[:])
                            nc.tensor.matmul(pw[:], wT[:], sel[:],
                                             start=(h == 0), stop=(h == 1))
                    if "wsel" in dbg:
                        wtmp = wk.tile([NU, BPC], dt.float32)
                        nc.vector.tensor_copy(wtmp[:], pwn[:])
                        nc.sync.dma_start(dbg["wsel"][0][:], wtmp[:])
                        wtmp2 = wk.tile([NU, BPC], dt.float32)
                        nc.vector.tensor_copy(wtmp2[:], pwd[:])
                        nc.sync.dma_start(dbg["wsel"][1][:], wtmp2[:])

                    # recurrence constants: nm_pre = wns + gleak*vleak,
                    # dcw = wds + cm_t + gleak
                    nm_pre = wk.tile([NU, BPC], dt.float32)
                    nc.vector.tensor_scalar(nm_pre[:], pwn[:], glvl[:], None,
                                            ALU.add)
                    dcw = wk.tile([NU, BPC], dt.float32)
                    nc.vector.tensor_scalar(dcw[:], pwd[:], dc[:], None, ALU.add)

                def finish_iter(pn, pd, v):
                    """Close psum groups with the constant injects, then
                    v' = (cm_t*v + pn) * 1/pd, all [NU, BPC]."""
                    nc.tensor.matmul(pn[:], eye32[:], nm_pre[:], start=False,
                                     stop=True)
                    nc.tensor.matmul(pd[:], eye32[:], dcw[:], start=False,
                                     stop=True)
                    num = wk.tile([NU, BPC], dt.float32, tag="num", name="num")
                    nc.vector.scalar_tensor_tensor(num[:], v[:], cmt[:], pn[:],
                                                   ALU.mult, ALU.add)
                    rden = wk.tile([NU, BPC], dt.float32, tag="rden", name="rden")
                    nc.vector.reciprocal(rden[:], pd[:])
                    vn = wk.tile([NU, BPC], dt.float32, tag="v", name="v")
                    nc.vector.tensor_tensor(vn[:], num[:], rden[:], ALU.mult)
                    return vn

                v = finish_iter(pn, pd, v)

                for k in range(1, UNFOLDS):
                    pV = psR.tile([128, BPC], dt.float32, tag="pV", name="pV")
                    nc.tensor.matmul(pV[:], rep4[:], v[:], start=True, stop=True)
                    sA = wk.tile([128, 8, BPC], dt.bfloat16, tag="sA", name="sA")
                    for jt in range(8):
                        nc.scalar.activation(sA[:, jt, :], pV[:], AF.Sigmoid,
                                             bias=msigv[:, jt:jt + 1],
                                             scale=sigv[:, jt:jt + 1])
                    pn = psR.tile([NU, BPC], dt.float32, tag="pn", name="pn")
                    pd = psR.tile([NU, BPC], dt.float32, tag="pd", name="pd")
                    for jt in range(8):
                        nc.tensor.matmul(pn[:], wseln[:, jt, :], sA[:, jt, :],
                                         start=(jt == 0), stop=False)
                    for jt in range(8):
                        nc.tensor.matmul(pd[:], wseld[:, jt, :], sA[:, jt, :],
                                         start=(jt == 0), stop=False)
                    v = finish_iter(pn, pd, v)

                if "h" in dbg:
                    nc.sync.dma_start(dbg["h"][:], v[:])

                # ---- cell B ----
                z2 = wk.tile([NU, BPC], dt.bfloat16)
                nc.vector.tensor_scalar(z2[:], v[:], ab[:], cb[:], ALU.mult,
                                        ALU.subtract)
                s2 = wk.tile([NU, BPC], dt.bfloat16)
                nc.scalar.activation(s2[:], z2[:], AF.Sigmoid)
                pbn = psR.tile([1, BPC], dt.float32, tag="pbn")
                pbd = psR.tile([1, BPC], dt.float32, tag="pbd")
                nc.tensor.matmul(pbn[:], w12b[:, 0:1], s2[:], start=True, stop=True)
                nc.tensor.matmul(pbd[:], w12b[:, 1:2], s2[:], start=True, stop=True)

                nm_preB = wk.tile([1, BPC], dt.float32)
                nc.vector.tensor_scalar(nm_preB[:], pbn[:], bsc[:, 5:6], None, ALU.add)
                dcwB = wk.tile([1, BPC], dt.float32)
                nc.vector.tensor_scalar(dcwB[:], pbd[:], bsc[:, 6:7], None, ALU.add)

                v2 = wk.tile([1, BPC], dt.float32)
                nc.vector.memset(v2[:], 0.0)
                for k in range(UNFOLDS):
                    sB = wk.tile([1, BPC], dt.float32)
                    nc.scalar.activation(sB[:], v2[:], AF.Sigmoid,
                                         bias=bsc[:, 1:2], scale=bsc[:, 0:1])
                    t1b = wk.tile([1, BPC], dt.float32)
                    nc.vector.scalar_tensor_tensor(t1b[:], v2[:], bsc[:, 4:5],
                                                   nm_preB[:], ALU.mult, ALU.add)
                    numB = wk.tile([1, BPC], dt.float32)
                    nc.vector.scalar_tensor_tensor(numB[:], sB[:], bsc[:, 2:3],
                                                   t1b[:], ALU.mult, ALU.add)
                    denB = wk.tile([1, BPC], dt.float32)
                    nc.vector.scalar_tensor_tensor(denB[:], sB[:], bsc[:, 3:4],
                                                   dcwB[:], ALU.mult, ALU.add)
                    rdenB = wk.tile([1, BPC], dt.float32)
                    nc.vector.reciprocal(rdenB[:], denB[:])
                    v2 = wk.tile([1, BPC], dt.float32)
                    nc.vector.tensor_tensor(v2[:], numB[:], rdenB[:], ALU.mult)

            outb = wk.tile([1, BPC], dt.float32)
            nc.scalar.activation(outb[:], v2[:], AF.Sigmoid)
            nc.sync.dma_start(d_out[:], outb[:])

    nc.compile()
    return nc


def prepare_inputs(inputs):
    """Host-side precompute: fold affines, build per-core input maps."""
    f32 = np.float32
    x = np.ascontiguousarray(inputs["x"]).reshape(B, NIN)

    # x pre-transposed + chunk-blocked: xq[ic, p, t, b] = xT[128*(4ic+t)+p, b]
    xT = np.ascontiguousarray(x.T)  # [NIN, B]
    xq = np.ascontiguousarray(
        xT.reshape(NCHUNK, ICG, 128, B).transpose(0, 2, 1, 3)).astype(BF16)

    iw, ib = f32(inputs["a_input_w"]), f32(inputs["a_input_b"])
    smu, ssig = f32(inputs["a_smu"]), f32(inputs["a_ssig"])
    sW, serev = f32(inputs["a_sW"]), f32(inputs["a_serev"])
    A = iw[:, None] * ssig                      # [NIN, NU]
    C = (smu - ib[:, None]) * ssig
    W1 = (sW * serev)
    W2 = sW

    # per-partition layout [128, it, u]
    def p_layout(m):  # [NIN, NU] -> [128, NIT, NU]
        return np.ascontiguousarray(m.reshape(NIT, 128, NU).transpose(1, 0, 2))

    Ap, Cp = p_layout(A), p_layout(-C)  # csc holds -C (both device paths add)
    W1p, W2p = p_layout(W1), p_layout(W2)

    # recurrence A params (global)
    mu, sig = f32(inputs["a_mu"]), f32(inputs["a_sig"])
    W, erev = f32(inputs["a_W"]), f32(inputs["a_erev"])
    gleak, vleak, cm = f32(inputs["a_gleak"]), f32(inputs["a_vleak"]), f32(inputs["a_cm"])
    cm_t = cm / np.float32(ELAPSED / UNFOLDS)
    Werev = W * erev

    # partition p = jb*32 + i within j-tile jt (j = 4*jt + jb)
    rep4 = np.zeros((NU, 128), f32)
    for p in range(128):
        rep4[p % NU, p] = 1.0
    sigv = np.zeros((128, 8), f32)
    msigv = np.zeros((128, 8), f32)
    wseln = np.zeros((128, 8, NU), f32)
    wseld = np.zeros((128, 8, NU), f32)
    for jt in range(8):
        for jb in range(4):
            j = 4 * jt + jb
            for i in range(NU):
                p = jb * NU + i
                sigv[p, jt] = sig[i, j]
                msigv[p, jt] = -mu[i, j] * sig[i, j]
                wseln[p, jt, j] = Werev[i, j]
                wseld[p, jt, j] = W[i, j]

    # cell B params
    iwb, ibb = f32(inputs["b_input_w"]), f32(inputs["b_input_b"])
    smub, ssigb = f32(inputs["b_smu"]), f32(inputs["b_ssig"])
    sWb, serevb = f32(inputs["b_sW"]), f32(inputs["b_serev"])
    Abv = (iwb[:, None] * ssigb)[:, 0]
    Cbv = ((smub - ibb[:, None]) * ssigb)[:, 0]
    w12b = np.stack([(sWb * serevb)[:, 0], sWb[:, 0]], axis=1)  # [NU, 2]
    mub, sigb_ = f32(inputs["b_mu"])[0, 0], f32(inputs["b_sig"])[0, 0]
    Wb_, erevb_ = f32(inputs["b_W"])[0, 0], f32(inputs["b_erev"])[0, 0]
    glb, vlb, cmb = f32(inputs["b_gleak"])[0], f32(inputs["b_vleak"])[0], f32(inputs["b_cm"])[0]
    cmtB = cmb / np.float32(ELAPSED / UNFOLDS)
    bsc = np.array([[sigb_, -mub * sigb_, Wb_ * erevb_, Wb_,
                     cmtB, glb * vlb, cmtB + glb, 0.0]], f32)

    common = dict(
        xq=xq,
        rep4=rep4,
        eye32=np.eye(NU, dtype=f32),
        wseln=wseln.astype(BF16), wseld=wseld.astype(BF16),
        sigv=sigv, msigv=msigv,
        cmt=cm_t.reshape(NU, 1), glvl=(gleak * vleak).reshape(NU, 1),
        dc=(cm_t + gleak).reshape(NU, 1),
        ab=Abv.reshape(NU, 1), cb=Cbv.reshape(NU, 1),
        w12b=w12b.astype(BF16), bsc=bsc,
    )

    in_maps = []
    for c in range(N_CORES):
        us = slice(UPC * c, UPC * (c + 1))
        w12c = np.zeros((128, NIT, UPC, 2 * UPC), f32)
        for u in range(UPC):
            w12c[:, :, u, 2 * u] = W1p[:, :, UPC * c + u]
            w12c[:, :, u, 2 * u + 1] = W2p[:, :, UPC * c + u]
        sel = np.zeros((2, 128, BPC), f32)
        for n in range(BPC):
            bg = BPC * c + n
            sel[bg // 128, bg % 128, n] = 1.0
        m = dict(common)
        m.update(
            asc=np.ascontiguousarray(Ap[:, :, us]),
            csc=np.ascontiguousarray(Cp[:, :, us]),
            w12=w12c.astype(BF16),
            selh=sel,
        )
        in_maps.append(m)
    return in_maps


_CACHED = {}


def kernel(**inputs):
    key = "prog"
    if key not in _CACHED:
        _CACHED[key] = build_program()
    nc = _CACHED[key]
    in_maps = prepare_inputs(inputs)
    res = run_bass_kernel_spmd(nc, in_maps, core_ids=list(range(N_CORES)))
    out = np.concatenate([res.results[c]["out"].reshape(BPC)
                          for c in range(N_CORES)])
    return out.astype(np.float32)


if __name__ == "__main__":
    d = np.load("/root/problem/ref_data.npz")
    inputs = {k: d[k] for k in d.files if k != "expected"}
    out = kernel(**inputs)
    exp = d["expected"]
    err = np.abs(out - exp)
    print("abs err max %.3e  rel err max %.3e" % (err.max(), (err / np.abs(exp)).max()))
